# revision 4
# baseline (speedup 1.0000x reference)
"""ConvLSTM decoder Trainium2 kernel, v2.

Strategy (v2 changes over v1)
-----------------------------
- Fully unrolled T=48 loop (no For_i back-edge barriers -> cross-step
  overlap; pointwise chains hide under the other layer's matmuls).
- Layer-1 x-side tap packing: x has only 64 channels, so two taps ride in
  one 128-partition contraction chunk. Host ships x2 = [x ; x shifted one
  col] and x3 = [x ; x shifted one row]; taps (dy,0)+(dy,1) pair via x2,
  (0,2)+(1,2) pair via x3, (2,2) runs alone on 64 partitions.
  L1: 9 h-taps + 5 x-passes = 14 passes/gate (was 18). L2: 18.
- Pair-blocked PSUM: per gate one [128, 4*512] f32 tile (4 banks, pair ip
  at col ip*512), drained by ONE activation over a strided [128,4,450] AP.
  2 tiles rotate (8 banks total).
- Emission order per layer: g0-self, g1-self, g0-in, g1-in, ACTs, then
  g2/g3 likewise, so layer-2's self (h2) taps run while layer-1's
  pointwise chain produces h1.
- Dense head: lhsT = h2 pixel slice [128c, 8img] (8-col weight loads),
  rhs = Wd slice [128c, 128o]; 225 matmuls N=128. Bias bd added on host.

Numerics: bf16 matmul operands, fp32 PSUM/gates/cell state (fp8 was
measured at 3-4% final error vs the 2% budget -- rejected).
"""

import numpy as np
import ml_dtypes

import concourse.bass as bass
from concourse import bacc
import concourse.mybir as mybir
import concourse.tile as tile
from concourse.bass_utils import run_bass_kernel_spmd

BF16 = ml_dtypes.bfloat16
F32 = mybir.dt.float32
BF = mybir.dt.bfloat16

B, T, C_IN, H, W = 64, 48, 64, 15, 15
HID, KK, OUT = 128, 3, 128
NCORES = 8
BC = B // NCORES          # images per core = 8
PH, PW = H + 2, W + 2     # padded plane 17x17
PP = PH * PW              # 289
ROW = BC * PP             # 2312
S = H * W                 # 225
NPAIR = BC // 2           # 4
NF = 2 * S                # 450 matmul free dim
NP1, NP2 = 14, 18         # passes per gate, layers 1 and 2
W1COLS = NP1 * 4 * 128    # 7168
W2COLS = NP2 * 4 * 128    # 9216
WDCOLS = S * OUT          # 28800
WTOT = W1COLS + W2COLS + WDCOLS
AFT = mybir.ActivationFunctionType


def build_nc(t_steps: int = T) -> bass.Bass:
    nc = bacc.Bacc("TRN2", target_bir_lowering=False, debug=False)

    x_d = nc.dram_tensor("x", [t_steps * 128, 2 * ROW], BF, kind="ExternalInput")
    w_d = nc.dram_tensor("w", [128, WTOT], BF, kind="ExternalInput")
    b_d = nc.dram_tensor("b", [128, 8], F32, kind="ExternalInput")
    out_d = nc.dram_tensor("out", [BC, OUT], F32, kind="ExternalOutput")

    h1p = nc.alloc_sbuf_tensor("h1p", [128, ROW], BF)
    h2p = nc.alloc_sbuf_tensor("h2p", [128, ROW], BF)
    c1 = nc.alloc_sbuf_tensor("c1", [128, BC * S], F32)
    c2 = nc.alloc_sbuf_tensor("c2", [128, BC * S], F32)
    wsb = nc.alloc_sbuf_tensor("wsb", [128, WTOT], BF)
    bsb = nc.alloc_sbuf_tensor("bsb", [128, 8], F32)

    def planes(ap):
        return ap.rearrange("p (i y x) -> p i y x", i=BC, y=PH, x=PW)

    wl1 = wsb.ap()[:, 0:W1COLS].rearrange(
        "p (ps g m) -> p ps g m", ps=NP1, g=4, m=128)
    wl2 = wsb.ap()[:, W1COLS:W1COLS + W2COLS].rearrange(
        "p (ps g m) -> p ps g m", ps=NP2, g=4, m=128)
    wdv = wsb.ap()[:, W1COLS + W2COLS:].rearrange(
        "p (s o) -> p s o", s=S, o=OUT)

    with tile.TileContext(nc) as tc:
        nc.vector.memset(h1p.ap()[:, :], 0.0)
        nc.vector.memset(h2p.ap()[:, :], 0.0)
        nc.vector.memset(c1.ap()[:, :], 0.0)
        nc.vector.memset(c2.ap()[:, :], 0.0)
        # Split the weight load so step-0 matmuls wait only on their slice.
        nc.sync.dma_start(wsb.ap()[:, 0:W1COLS], w_d.ap()[:, 0:W1COLS])
        nc.sync.dma_start(wsb.ap()[:, W1COLS:], w_d.ap()[:, W1COLS:])
        nc.sync.dma_start(bsb.ap()[:, :], b_d.ap()[:, :])

        with (
            tc.tile_pool(name="psum", bufs=2, space="PSUM") as psum,
            tc.tile_pool(name="gates", bufs=5) as gates,
            tc.tile_pool(name="tmps", bufs=2) as tmps,
            tc.tile_pool(name="xin", bufs=2) as xin,
        ):
            h1v, h2v = planes(h1p.ap()), planes(h2p.ap())

            def l1_passes(xt):
                """(lhsT-pass-idx, rhs_window_fn, n_part) for layer 1."""
                x2 = xt[:, 0:ROW].rearrange(
                    "p (i y x) -> p i y x", i=BC, y=PH, x=PW)
                x3 = xt[:, ROW:2 * ROW].rearrange(
                    "p (i y x) -> p i y x", i=BC, y=PH, x=PW)
                ps = []
                for tap in range(9):
                    dy, dx = divmod(tap, 3)
                    ps.append((lambda ip, dy=dy, dx=dx:
                               h1v[:, 2 * ip:2 * ip + 2, dy:dy + H, dx:dx + W],
                               128))
                for dy in range(3):
                    ps.append((lambda ip, dy=dy:
                               x2[:, 2 * ip:2 * ip + 2, dy:dy + H, 0:W], 128))
                ps.append((lambda ip: x3[:, 2 * ip:2 * ip + 2, 0:H, 2:2 + W],
                           128))
                ps.append((lambda ip: x2[0:64, 2 * ip:2 * ip + 2, 2:2 + H,
                                         2:2 + W], 64))
                return ps

            def l2_passes():
                ps = []
                for src in (h2v, h1v):
                    for tap in range(9):
                        dy, dx = divmod(tap, 3)
                        ps.append((lambda ip, dy=dy, dx=dx, src=src:
                                   src[:, 2 * ip:2 * ip + 2, dy:dy + H,
                                       dx:dx + W], 128))
                return ps

            def gate_mms(wv, passes, g, skip):
                """skip: drop pass indices in `skip` (zero contributions at
                t=0 when the hidden states are still all-zero)."""
                pt = psum.tile([128, 4 * 512], F32, tag="ps", name=f"ps{g}")
                live = [pi for pi in range(len(passes)) if pi not in skip]
                for j, pi in enumerate(live):
                    rhs_fn, npart = passes[pi]
                    lhsT = wv[0:npart, pi, g, :]
                    for ip in range(NPAIR):
                        nc.tensor.matmul(
                            pt[:, ip * 512:ip * 512 + NF], lhsT, rhs_fn(ip),
                            start=(j == 0), stop=(j == len(live) - 1))
                return pt

            def gate_act(pt, g, bofs):
                gt = gates.tile([128, BC * S], F32, tag="gate", name=f"g{g}")
                func = AFT.Tanh if g == 3 else AFT.Sigmoid
                nc.scalar.activation(
                    gt.rearrange("p (i n) -> p i n", i=NPAIR),
                    pt.rearrange("p (i n) -> p i n", i=NPAIR)[:, :, 0:NF],
                    func, bias=bsb.ap()[:, bofs + g:bofs + g + 1])
                return gt

            def lstm_layer(passes, wv, bofs, cst, selfv, skip=()):
                gsb = [None] * 4
                for gpair in ((0, 1), (2, 3)):
                    pts = {g: gate_mms(wv, passes, g, skip) for g in gpair}
                    for g in gpair:
                        gsb[g] = gate_act(pts[g], g, bofs)
                gi, gf, go, gg = gsb
                t1 = tmps.tile([128, BC * S], F32, tag="tmp", name="t1")
                t2 = tmps.tile([128, BC * S], F32, tag="tmp", name="t2")
                nc.vector.tensor_mul(t1[:, :], gf[:, :], cst.ap()[:, :])
                nc.vector.tensor_mul(t2[:, :], gi[:, :], gg[:, :])
                nc.vector.tensor_add(cst.ap()[:, :], t1[:, :], t2[:, :])
                tch = tmps.tile([128, BC * S], F32, tag="tmp", name="tch")
                nc.scalar.activation(tch[:, :], cst.ap()[:, :], AFT.Tanh)
                hdst = selfv[:, :, 1:1 + H, 1:1 + W]
                ov = go[:, :].rearrange("p (i y x) -> p i y x", i=BC, y=H, x=W)
                tv = tch[:, :].rearrange("p (i y x) -> p i y x", i=BC, y=H, x=W)
                nc.vector.tensor_mul(hdst, ov, tv)

            ps2 = l2_passes()
            for t in range(t_steps):
                xt = xin.tile([128, 2 * ROW], BF, tag="x", name=f"x{t}")
                nc.sync.dma_start(xt[:, :],
                                  x_d.ap()[t * 128:(t + 1) * 128, :])
                # At t=0 both hidden states are all-zero: their conv taps
                # contribute nothing, so skip those passes entirely.
                zskip = tuple(range(9)) if t == 0 else ()
                lstm_layer(l1_passes(xt), wl1, 0, c1, h1v, skip=zskip)
                lstm_layer(ps2, wl2, 4, c2, h2v, skip=zskip)

        # Dense head: out[img, o] = sum_{c,s} h2[c, img, s] * Wd[(c,s), o]
        with (
            tc.tile_pool(name="psum2", bufs=1, space="PSUM") as psum2,
            tc.tile_pool(name="outp", bufs=1) as outp,
        ):
            po = psum2.tile([BC, OUT], F32, tag="po", name="po")
            for s in range(S):
                py, px = divmod(s, W)
                lhsT = h2v[:, :, 1 + py, 1 + px]
                nc.tensor.matmul(po[:, :], lhsT, wdv[:, s, :],
                                 start=(s == 0), stop=(s == S - 1))
            osb = outp.tile([BC, OUT], F32, tag="o", name="osb")
            nc.vector.tensor_copy(osb[:, :], po[:, :])
            nc.sync.dma_start(out_d.ap()[:, :], osb[:, :])

    nc.compile()
    return nc


def pack_inputs(inputs: dict, t_steps: int = T) -> tuple[list[dict], dict]:
    enc = np.ascontiguousarray(np.asarray(inputs["encoder_output"], np.float32))
    W0 = np.asarray(inputs["W0"], np.float32)
    W1 = np.asarray(inputs["W1"], np.float32)
    b0 = np.asarray(inputs["b0"], np.float32)
    b1 = np.asarray(inputs["b1"], np.float32)
    Wd = np.asarray(inputs["Wd"], np.float32)

    # --- weights ---
    # layer 1: wl1[c, pass, gate, m]; gate g covers out channels g*128+m
    wx = W0[:, :C_IN]          # [512, 64, 3, 3]
    wh = W0[:, C_IN:]          # [512, 128, 3, 3]
    w1 = np.zeros((128, NP1, 4, 128), np.float32)
    W0g = wh.reshape(4, 128, 128, 3, 3)     # [g, m, c, dy, dx]
    Wxg = wx.reshape(4, 128, C_IN, 3, 3)
    for tap in range(9):
        dy, dx = divmod(tap, 3)
        w1[:, tap] = W0g[:, :, :, dy, dx].transpose(2, 0, 1)
    for dy in range(3):
        w1[0:64, 9 + dy] = Wxg[:, :, :, dy, 0].transpose(2, 0, 1)
        w1[64:128, 9 + dy] = Wxg[:, :, :, dy, 1].transpose(2, 0, 1)
    w1[0:64, 12] = Wxg[:, :, :, 0, 2].transpose(2, 0, 1)
    w1[64:128, 12] = Wxg[:, :, :, 1, 2].transpose(2, 0, 1)
    w1[0:64, 13] = Wxg[:, :, :, 2, 2].transpose(2, 0, 1)

    # layer 2: self (h2) taps 0-8 then input (h1) taps 9-17
    wh1 = W1[:, :HID].reshape(4, 128, 128, 3, 3)
    wh2 = W1[:, HID:].reshape(4, 128, 128, 3, 3)
    w2 = np.zeros((128, NP2, 4, 128), np.float32)
    for tap in range(9):
        dy, dx = divmod(tap, 3)
        w2[:, tap] = wh2[:, :, :, dy, dx].transpose(2, 0, 1)
        w2[:, 9 + tap] = wh1[:, :, :, dy, dx].transpose(2, 0, 1)

    wall = np.concatenate(
        [w1.reshape(128, W1COLS), w2.reshape(128, W2COLS),
         Wd.reshape(HID, S * OUT)], axis=1).astype(BF16)
    ball = np.concatenate(
        [b0.reshape(4, 128).T, b1.reshape(4, 128).T], axis=1).astype(np.float32)
    ball = np.ascontiguousarray(ball)

    shared = {"w": wall, "b": ball}
    in_maps = []
    for c in range(NCORES):
        xc = enc[c * BC:(c + 1) * BC, :t_steps]      # [8, t, 64, 15, 15]
        xp = np.zeros((t_steps, C_IN, BC, PH, PW), np.float32)
        xp[:, :, :, 1:1 + H, 1:1 + W] = xc.transpose(1, 2, 0, 3, 4)
        xfull = np.zeros((t_steps, 128, 2, BC, PH, PW), np.float32)
        xfull[:, 0:64, 0] = xp                       # x2 top: x
        xfull[:, 64:128, 0, :, :, :-1] = xp[..., 1:]  # x2 bot: col-shift
        xfull[:, 0:64, 1] = xp                       # x3 top: x
        xfull[:, 64:128, 1, :, :-1, :] = xp[:, :, :, 1:, :]  # x3 bot: row-shift
        in_maps.append({"x": xfull.astype(BF16).reshape(t_steps * 128, 2 * ROW),
                        **shared})
    return in_maps, shared


def unpack_output(results, inputs) -> np.ndarray:
    bd = np.asarray(inputs["bd"], np.float32)
    out = np.concatenate(
        [np.asarray(r["out"], np.float32) for r in results], axis=0)
    return np.ascontiguousarray(out + bd[None, :])


def kernel(**inputs) -> np.ndarray:
    nc = build_nc(T)
    in_maps, _ = pack_inputs(inputs, T)
    res = run_bass_kernel_spmd(nc, in_maps, list(range(NCORES))).results
    return unpack_output(res, inputs)
